# revision 6
# baseline (speedup 1.0000x reference)
"""Trainium2 Bass kernel for nn_MOLELinear (MoE-style mixed linear layer).

Math (per graph g):
    mixed_w[g] = sum_e coefficients[g, e] * weight_experts[e] + weight_shared[0]
    mixed_b[g] = coefficients[g] @ bias_experts + bias_shared[0]
    out[g]     = x[g] @ mixed_w[g].T + mixed_b[g]

Strategy (8 NeuronCores, data-parallel over graphs; 8 graphs per core):
  * The mix einsum is 0.8% of total FLOPs, and shipping premixed per-graph
    weights costs exactly the same DMA bytes as shipping the expert stack
    (16.8 MB/core either way). So the host premixes mixed_w (and mixed_b),
    pre-transposes both mixed_w and x into the [i-major] operand layout the
    PE needs, and casts to fp16. The device then runs ONLY the main matmul:
    dense back-to-back K=128/M=128/N=512 fp16 matmuls accumulating over 8
    i-blocks in PSUM — no transposes, no mix phase, PE stays HAM-warm.
  * Ramp hiding: graph 0's operands arrive as 16 per-i-block chunk DMAs and
    six PSUM accumulation groups (rb 0-2 x oc 0-1) are interleaved
    chunk-by-chunk, so the PE computes during the initial HBM transfer
    instead of idling ~17 us.
  * Bias: premixed on host, broadcast across partitions on-device with a
    K=1 matmul against a ones row, added during PSUM->SBUF evacuation by
    DVE tensor_tensor.
  * Output is written fp16 (halves output DMA; ~2e-4 extra rel err) and
    cast back to f32 on the host. Output DMAs ride the ACT HWDGE ring so
    they don't queue behind input DMAs on Sync.
"""

import numpy as np

import concourse.bacc as bacc
import concourse.mybir as mybir
import concourse.tile as tile
from concourse.bass_utils import run_bass_kernel_spmd

f32 = mybir.dt.float32
fp16 = mybir.dt.float16

NCORES = 8
G = 64                  # total graphs
GPC = G // NCORES       # graphs per core
R = 1024                # rows per graph
IN_F = 1024
OUT_F = 1024
E = 8                   # routed experts
NIB = IN_F // 128       # i blocks (8)
NRB = R // 128          # row blocks per graph (8)

_CACHED = {}


def build_kernel():
    nc = bacc.Bacc(None, target_bir_lowering=False)

    # host-premixed, transposed operands (SBUF layout, contiguous rows):
    #   mt[g*128+p, ib*OUT_F+o] = mixed_w[g][o, ib*128+p]
    #   xt[g*128+p, ib*R + r]   = x[g*R+r, ib*128+p]
    mt_ext = nc.declare_dram_parameter("mt", [GPC * 128, NIB * OUT_F], fp16,
                                       isOutput=False)
    xt_ext = nc.declare_dram_parameter("xt", [GPC * 128, NIB * R], fp16,
                                       isOutput=False)
    bmix_ext = nc.declare_dram_parameter("bmix", [1, GPC * OUT_F], fp16,
                                         isOutput=False)
    ones_ext = nc.declare_dram_parameter("ones", [1, 128], fp16, isOutput=False)
    out_ext = nc.declare_dram_parameter("out", [GPC * R, OUT_F], fp16,
                                        isOutput=True)

    with tile.TileContext(nc) as tc:
        with (
            tc.tile_pool(name="consts", bufs=1) as cpool,
            tc.tile_pool(name="mt", bufs=2) as mtpool,
            tc.tile_pool(name="xt", bufs=2) as xtpool,
            tc.tile_pool(name="outs", bufs=4) as opool,
            tc.tile_pool(name="brep", bufs=2) as bpool,
            tc.tile_pool(name="psB", bufs=2, space="PSUM") as psB,
            tc.tile_pool(name="psC", bufs=6, space="PSUM") as psC,
        ):
            ones_t = cpool.tile([1, 128], fp16, tag="ones")
            bmix_t = cpool.tile([1, GPC * OUT_F], fp16, tag="bmix")
            nc.scalar.dma_start(out=ones_t[:], in_=ones_ext[:])
            nc.scalar.dma_start(out=bmix_t[:], in_=bmix_ext[:])

            def make_brep(g):
                brep_t = bpool.tile([128, OUT_F], f32, tag="brep")
                for oc in range(2):
                    b_ps = psB.tile([128, 512], f32, tag="bps")
                    nc.tensor.matmul(
                        b_ps[:], ones_t[:],
                        bmix_t[:, g * OUT_F + oc * 512:g * OUT_F + (oc + 1) * 512],
                        start=True, stop=True)
                    nc.scalar.copy(brep_t[:, oc * 512:(oc + 1) * 512], b_ps[:])
                return brep_t

            def mm_group(ps, xt_t, mt_t, rb, oc, ib, skip=False):
                nc.tensor.matmul(
                    ps[:],
                    xt_t[:, ib * R + rb * 128:ib * R + (rb + 1) * 128],
                    mt_t[:, ib * OUT_F + oc * 512:ib * OUT_F + (oc + 1) * 512],
                    start=(ib == 0), stop=(ib == NIB - 1),
                    skip_group_check=skip,
                )

            def evac_and_store(g, rb, ps_pair, brep_t):
                out_sb = opool.tile([128, OUT_F], fp16, tag="osb")
                for oc in range(2):
                    nc.vector.tensor_tensor(
                        out=out_sb[:, oc * 512:(oc + 1) * 512],
                        in0=ps_pair[oc][:],
                        in1=brep_t[:, oc * 512:(oc + 1) * 512],
                        op=mybir.AluOpType.add,
                    )
                nc.scalar.dma_start(
                    out=out_ext[g * R + rb * 128:g * R + (rb + 1) * 128, :],
                    in_=out_sb[:],
                )

            # ---- graph 0: chunked streaming with 6-way group interleave ----
            mt_t = mtpool.tile([128, NIB * OUT_F], fp16, tag="mt")
            xt_t = xtpool.tile([128, NIB * R], fp16, tag="xt")
            for ib in range(NIB):
                nc.sync.dma_start(
                    out=mt_t[:, ib * OUT_F:(ib + 1) * OUT_F],
                    in_=mt_ext[0:128, ib * OUT_F:(ib + 1) * OUT_F])
                nc.scalar.dma_start(
                    out=xt_t[:, ib * R:(ib + 1) * R],
                    in_=xt_ext[0:128, ib * R:(ib + 1) * R])

            brep_t = make_brep(0)
            ps6 = [psC.tile([128, 512], f32, tag="outps", name=f"ps6_{j}")
                   for j in range(6)]
            for ib in range(NIB):
                for j in range(6):
                    mm_group(ps6[j], xt_t, mt_t, j // 2, j % 2, ib, skip=True)
            for rb in range(3):
                evac_and_store(0, rb, ps6[rb * 2:rb * 2 + 2], brep_t)
            for rb in range(3, NRB):
                ps_pair = []
                for oc in range(2):
                    ps = psC.tile([128, 512], f32, tag="outps")
                    for ib in range(NIB):
                        mm_group(ps, xt_t, mt_t, rb, oc, ib)
                    ps_pair.append(ps)
                evac_and_store(0, rb, ps_pair, brep_t)

            # ---- graphs 1..GPC-1: steady-state pipeline ----
            for g in range(1, GPC):
                mt_t = mtpool.tile([128, NIB * OUT_F], fp16, tag="mt")
                nc.sync.dma_start(out=mt_t[:], in_=mt_ext[g * 128:(g + 1) * 128, :])
                xt_t = xtpool.tile([128, NIB * R], fp16, tag="xt")
                nc.sync.dma_start(out=xt_t[:], in_=xt_ext[g * 128:(g + 1) * 128, :])

                brep_t = make_brep(g)
                for rb in range(NRB):
                    ps_pair = []
                    for oc in range(2):
                        ps = psC.tile([128, 512], f32, tag="outps")
                        for ib in range(NIB):
                            mm_group(ps, xt_t, mt_t, rb, oc, ib)
                        ps_pair.append(ps)
                    evac_and_store(g, rb, ps_pair, brep_t)
    nc.compile()
    return nc


def _host_prep(x, coefficients, weight_experts, bias_experts, weight_shared,
               bias_shared):
    c32 = coefficients.astype(np.float32)
    # mixed weights [G, O, I] in f32, then to [G, 128(p), NIB, O] fp16
    mw = (c32 @ weight_experts.reshape(E, -1).astype(np.float32)).reshape(
        G, OUT_F, IN_F)
    mw += weight_shared[0]
    # mt[g, p, ib, o] = mw[g, o, ib*128+p]
    mt = np.ascontiguousarray(
        mw.reshape(G, OUT_F, NIB, 128).transpose(0, 3, 2, 1)).astype(np.float16)
    mt = mt.reshape(G * 128, NIB * OUT_F)

    # xt[g, p, ib, r] = x[g*R+r, ib*128+p]
    xt = np.ascontiguousarray(
        x.reshape(G, R, NIB, 128).transpose(0, 3, 2, 1)).astype(np.float16)
    xt = xt.reshape(G * 128, NIB * R)

    bm = (c32 @ bias_experts.astype(np.float32) + bias_shared[0]).astype(
        np.float16)  # [G, OUT_F]
    ones = np.ones((1, 128), dtype=np.float16)

    in_maps = []
    for c in range(NCORES):
        in_maps.append({
            "mt": mt[c * GPC * 128:(c + 1) * GPC * 128],
            "xt": xt[c * GPC * 128:(c + 1) * GPC * 128],
            "bmix": bm[c * GPC:(c + 1) * GPC].reshape(1, GPC * OUT_F),
            "ones": ones,
        })
    return in_maps


def kernel(x, coefficients, weight_experts, bias_experts, weight_shared,
           bias_shared, _want_trace=False):
    if "nc" not in _CACHED:
        _CACHED["nc"] = build_kernel()
    nc = _CACHED["nc"]
    in_maps = _host_prep(x, coefficients, weight_experts, bias_experts,
                         weight_shared, bias_shared)
    kw = {}
    if _want_trace:
        kw = dict(trace=True)
    res = run_bass_kernel_spmd(nc, in_maps, core_ids=list(range(NCORES)), **kw)
    _CACHED["last_result"] = res
    out = np.concatenate(
        [res.results[c]["out"] for c in range(NCORES)], axis=0
    ).astype(np.float32)
    return out


# revision 8
# speedup vs baseline: 1.0205x; 1.0205x over previous
"""Trainium2 Bass kernel for nn_MOLELinear (MoE-style mixed linear layer).

Math (per graph g):
    mixed_w[g] = sum_e coefficients[g, e] * weight_experts[e] + weight_shared[0]
    mixed_b[g] = coefficients[g] @ bias_experts + bias_shared[0]
    out[g]     = x[g] @ mixed_w[g].T + mixed_b[g]

Strategy (8 NeuronCores, data-parallel over graphs; 8 graphs per core):
  * The mix einsum is 0.8% of total FLOPs, and shipping premixed per-graph
    weights costs exactly the same DMA bytes as shipping the expert stack
    (16.8 MB/core either way). So the host premixes mixed_w (and mixed_b),
    pre-transposes both mixed_w and x into the [i-major] operand layout the
    PE needs, and casts to fp16. The device then runs ONLY the main matmul:
    dense back-to-back K=128/M=128/N=512 fp16 matmuls accumulating over 8
    i-blocks in PSUM — no transposes, no mix phase, PE stays HAM-warm.
  * Ramp hiding: graph 0's operands arrive as 16 per-i-block chunk DMAs and
    six PSUM accumulation groups (rb 0-2 x oc 0-1) are interleaved
    chunk-by-chunk, so the PE computes during the initial HBM transfer
    instead of idling ~17 us.
  * Bias: premixed on host, broadcast across partitions on-device with a
    K=1 matmul against a ones row, added during PSUM->SBUF evacuation by
    DVE tensor_tensor.
  * Output is written fp16 (halves output DMA; ~2e-4 extra rel err) and
    cast back to f32 on the host. Output DMAs ride the ACT HWDGE ring so
    they don't queue behind input DMAs on Sync.
"""

import numpy as np

import concourse.bacc as bacc
import concourse.mybir as mybir
import concourse.tile as tile
from concourse.bass_utils import run_bass_kernel_spmd

f32 = mybir.dt.float32
fp16 = mybir.dt.float16

NCORES = 8
G = 64                  # total graphs
GPC = G // NCORES       # graphs per core
R = 1024                # rows per graph
IN_F = 1024
OUT_F = 1024
E = 8                   # routed experts
NIB = IN_F // 128       # i blocks (8)
NRB = R // 128          # row blocks per graph (8)

_CACHED = {}


def build_kernel():
    nc = bacc.Bacc(None, target_bir_lowering=False)

    # host-premixed, transposed operands (SBUF layout, contiguous rows):
    #   mt[g*128+p, ib*OUT_F+o] = mixed_w[g][o, ib*128+p]
    #   xt[g*128+p, ib*R + r]   = x[g*R+r, ib*128+p]
    mt_ext = nc.declare_dram_parameter("mt", [GPC * 128, NIB * OUT_F], fp16,
                                       isOutput=False)
    xt_ext = nc.declare_dram_parameter("xt", [GPC * 128, NIB * R], fp16,
                                       isOutput=False)
    bmix_ext = nc.declare_dram_parameter("bmix", [1, GPC * OUT_F], fp16,
                                         isOutput=False)
    ones_ext = nc.declare_dram_parameter("ones", [1, 128], fp16, isOutput=False)
    out_ext = nc.declare_dram_parameter("out", [GPC * R, OUT_F], fp16,
                                        isOutput=True)

    with tile.TileContext(nc) as tc:
        with (
            tc.tile_pool(name="consts", bufs=1) as cpool,
            tc.tile_pool(name="mt", bufs=2) as mtpool,
            tc.tile_pool(name="xt", bufs=2) as xtpool,
            tc.tile_pool(name="outs", bufs=4) as opool,
            tc.tile_pool(name="brep", bufs=2) as bpool,
            tc.tile_pool(name="psB", bufs=2, space="PSUM") as psB,
            tc.tile_pool(name="psC", bufs=6, space="PSUM") as psC,
        ):
            ones_t = cpool.tile([1, 128], fp16, tag="ones")
            bmix_t = cpool.tile([1, GPC * OUT_F], fp16, tag="bmix")

            def make_brep(g):
                brep_t = bpool.tile([128, OUT_F], f32, tag="brep")
                for oc in range(2):
                    b_ps = psB.tile([128, 512], f32, tag="bps")
                    nc.tensor.matmul(
                        b_ps[:], ones_t[:],
                        bmix_t[:, g * OUT_F + oc * 512:g * OUT_F + (oc + 1) * 512],
                        start=True, stop=True)
                    nc.scalar.copy(brep_t[:, oc * 512:(oc + 1) * 512], b_ps[:])
                return brep_t

            def mm_group(ps, xt_t, mt_t, rb, oc, ib, skip=False):
                nc.tensor.matmul(
                    ps[:],
                    xt_t[:, ib * R + rb * 128:ib * R + (rb + 1) * 128],
                    mt_t[:, ib * OUT_F + oc * 512:ib * OUT_F + (oc + 1) * 512],
                    start=(ib == 0), stop=(ib == NIB - 1),
                    skip_group_check=skip,
                )

            def evac_and_store(g, rb, ps_pair, brep_t):
                out_sb = opool.tile([128, OUT_F], fp16, tag="osb")
                for oc in range(2):
                    nc.vector.tensor_tensor(
                        out=out_sb[:, oc * 512:(oc + 1) * 512],
                        in0=ps_pair[oc][:],
                        in1=brep_t[:, oc * 512:(oc + 1) * 512],
                        op=mybir.AluOpType.add,
                    )
                nc.scalar.dma_start(
                    out=out_ext[g * R + rb * 128:g * R + (rb + 1) * 128, :],
                    in_=out_sb[:],
                )

            # ---- graph 0: chunked streaming with 6-way group interleave ----
            mt_t = mtpool.tile([128, NIB * OUT_F], fp16, tag="mt")
            xt_t = xtpool.tile([128, NIB * R], fp16, tag="xt")
            for ib in range(NIB):
                nc.sync.dma_start(
                    out=mt_t[:, ib * OUT_F:(ib + 1) * OUT_F],
                    in_=mt_ext[0:128, ib * OUT_F:(ib + 1) * OUT_F])
                nc.sync.dma_start(
                    out=xt_t[:, ib * R:(ib + 1) * R],
                    in_=xt_ext[0:128, ib * R:(ib + 1) * R])
            nc.sync.dma_start(out=ones_t[:], in_=ones_ext[:])
            nc.sync.dma_start(out=bmix_t[:], in_=bmix_ext[:])

            ps6 = [psC.tile([128, 512], f32, tag="outps", name=f"ps6_{j}")
                   for j in range(6)]
            for ib in range(NIB):
                for j in range(6):
                    mm_group(ps6[j], xt_t, mt_t, j // 2, j % 2, ib, skip=True)
            brep_t = make_brep(0)
            for rb in range(3):
                evac_and_store(0, rb, ps6[rb * 2:rb * 2 + 2], brep_t)
            for rb in range(3, NRB):
                ps_pair = []
                for oc in range(2):
                    ps = psC.tile([128, 512], f32, tag="outps")
                    for ib in range(NIB):
                        mm_group(ps, xt_t, mt_t, rb, oc, ib)
                    ps_pair.append(ps)
                evac_and_store(0, rb, ps_pair, brep_t)

            # ---- graphs 1..GPC-1: steady-state pipeline ----
            for g in range(1, GPC):
                mt_t = mtpool.tile([128, NIB * OUT_F], fp16, tag="mt")
                nc.sync.dma_start(out=mt_t[:], in_=mt_ext[g * 128:(g + 1) * 128, :])
                xt_t = xtpool.tile([128, NIB * R], fp16, tag="xt")
                nc.sync.dma_start(out=xt_t[:], in_=xt_ext[g * 128:(g + 1) * 128, :])

                brep_t = make_brep(g)
                for rb in range(NRB):
                    ps_pair = []
                    for oc in range(2):
                        ps = psC.tile([128, 512], f32, tag="outps")
                        for ib in range(NIB):
                            mm_group(ps, xt_t, mt_t, rb, oc, ib)
                        ps_pair.append(ps)
                    evac_and_store(g, rb, ps_pair, brep_t)
    nc.compile()
    return nc


def _host_prep(x, coefficients, weight_experts, bias_experts, weight_shared,
               bias_shared):
    c32 = coefficients.astype(np.float32)
    # mixed weights [G, O, I] in f32, then to [G, 128(p), NIB, O] fp16
    mw = (c32 @ weight_experts.reshape(E, -1).astype(np.float32)).reshape(
        G, OUT_F, IN_F)
    mw += weight_shared[0]
    # mt[g, p, ib, o] = mw[g, o, ib*128+p]
    mt = np.ascontiguousarray(
        mw.reshape(G, OUT_F, NIB, 128).transpose(0, 3, 2, 1)).astype(np.float16)
    mt = mt.reshape(G * 128, NIB * OUT_F)

    # xt[g, p, ib, r] = x[g*R+r, ib*128+p]
    xt = np.ascontiguousarray(
        x.reshape(G, R, NIB, 128).transpose(0, 3, 2, 1)).astype(np.float16)
    xt = xt.reshape(G * 128, NIB * R)

    bm = (c32 @ bias_experts.astype(np.float32) + bias_shared[0]).astype(
        np.float16)  # [G, OUT_F]
    ones = np.ones((1, 128), dtype=np.float16)

    in_maps = []
    for c in range(NCORES):
        in_maps.append({
            "mt": mt[c * GPC * 128:(c + 1) * GPC * 128],
            "xt": xt[c * GPC * 128:(c + 1) * GPC * 128],
            "bmix": bm[c * GPC:(c + 1) * GPC].reshape(1, GPC * OUT_F),
            "ones": ones,
        })
    return in_maps


def kernel(x, coefficients, weight_experts, bias_experts, weight_shared,
           bias_shared, _want_trace=False):
    if "nc" not in _CACHED:
        _CACHED["nc"] = build_kernel()
    nc = _CACHED["nc"]
    in_maps = _host_prep(x, coefficients, weight_experts, bias_experts,
                         weight_shared, bias_shared)
    kw = {}
    if _want_trace:
        kw = dict(trace=True)
    res = run_bass_kernel_spmd(nc, in_maps, core_ids=list(range(NCORES)), **kw)
    _CACHED["last_result"] = res
    out = np.concatenate(
        [res.results[c]["out"] for c in range(NCORES)], axis=0
    ).astype(np.float32)
    return out


# revision 9
# speedup vs baseline: 1.0244x; 1.0039x over previous
"""Trainium2 Bass kernel for nn_MOLELinear (MoE-style mixed linear layer).

Math (per graph g):
    mixed_w[g] = sum_e coefficients[g, e] * weight_experts[e] + weight_shared[0]
    mixed_b[g] = coefficients[g] @ bias_experts + bias_shared[0]
    out[g]     = x[g] @ mixed_w[g].T + mixed_b[g]

Strategy (8 NeuronCores, data-parallel over graphs; 8 graphs per core):
  * The mix einsum is 0.8% of total FLOPs, and shipping premixed per-graph
    weights costs exactly the same DMA bytes as shipping the expert stack
    (16.8 MB/core either way). So the host premixes mixed_w (and mixed_b),
    pre-transposes both mixed_w and x into the [i-major] operand layout the
    PE needs, and casts to fp16. The device then runs ONLY the main matmul:
    dense back-to-back K=128/M=128/N=512 fp16 matmuls accumulating over 8
    i-blocks in PSUM — no transposes, no mix phase, PE stays HAM-warm.
  * Ramp hiding: graph 0's operands arrive as 16 per-i-block chunk DMAs and
    six PSUM accumulation groups (rb 0-2 x oc 0-1) are interleaved
    chunk-by-chunk, so the PE computes during the initial HBM transfer
    instead of idling ~17 us.
  * Bias: premixed on host, broadcast across partitions on-device with a
    K=1 matmul against a ones row, added during PSUM->SBUF evacuation by
    DVE tensor_tensor.
  * Output is written fp16 (halves output DMA; ~2e-4 extra rel err) and
    cast back to f32 on the host. Output DMAs ride the ACT HWDGE ring so
    they don't queue behind input DMAs on Sync.
"""

import numpy as np

import concourse.bacc as bacc
import concourse.mybir as mybir
import concourse.tile as tile
from concourse.bass_utils import run_bass_kernel_spmd

f32 = mybir.dt.float32
fp16 = mybir.dt.float16

NCORES = 8
G = 64                  # total graphs
GPC = G // NCORES       # graphs per core
R = 1024                # rows per graph
IN_F = 1024
OUT_F = 1024
E = 8                   # routed experts
NIB = IN_F // 128       # i blocks (8)
NRB = R // 128          # row blocks per graph (8)

_CACHED = {}


def build_kernel():
    nc = bacc.Bacc(None, target_bir_lowering=False)

    # host-premixed, transposed operands (SBUF layout, contiguous rows):
    #   mt[g*128+p, ib*OUT_F+o] = mixed_w[g][o, ib*128+p]
    #   xt[g*128+p, ib*R + r]   = x[g*R+r, ib*128+p]
    mt_ext = nc.declare_dram_parameter("mt", [GPC * 128, NIB * OUT_F], fp16,
                                       isOutput=False)
    xt_ext = nc.declare_dram_parameter("xt", [GPC * 128, NIB * R], fp16,
                                       isOutput=False)
    bmix_ext = nc.declare_dram_parameter("bmix", [1, GPC * OUT_F], fp16,
                                         isOutput=False)
    ones_ext = nc.declare_dram_parameter("ones", [1, 128], fp16, isOutput=False)
    out_ext = nc.declare_dram_parameter("out", [GPC * R, OUT_F], fp16,
                                        isOutput=True)

    with tile.TileContext(nc) as tc:
        with (
            tc.tile_pool(name="consts", bufs=1) as cpool,
            tc.tile_pool(name="mt", bufs=2) as mtpool,
            tc.tile_pool(name="xt", bufs=2) as xtpool,
            tc.tile_pool(name="outs", bufs=4) as opool,
            tc.tile_pool(name="brep", bufs=2) as bpool,
            tc.tile_pool(name="psB", bufs=2, space="PSUM") as psB,
            tc.tile_pool(name="psC", bufs=6, space="PSUM") as psC,
        ):
            ones_t = cpool.tile([1, 128], fp16, tag="ones")
            bmix_t = cpool.tile([1, GPC * OUT_F], fp16, tag="bmix")

            def make_brep(g):
                brep_t = bpool.tile([128, OUT_F], f32, tag="brep")
                for oc in range(2):
                    b_ps = psB.tile([128, 512], f32, tag="bps")
                    nc.tensor.matmul(
                        b_ps[:], ones_t[:],
                        bmix_t[:, g * OUT_F + oc * 512:g * OUT_F + (oc + 1) * 512],
                        start=True, stop=True)
                    nc.scalar.copy(brep_t[:, oc * 512:(oc + 1) * 512], b_ps[:])
                return brep_t

            def mm_group(ps, xt_t, mt_t, rb, oc, ib, skip=False):
                nc.tensor.matmul(
                    ps[:],
                    xt_t[:, ib * R + rb * 128:ib * R + (rb + 1) * 128],
                    mt_t[:, ib * OUT_F + oc * 512:ib * OUT_F + (oc + 1) * 512],
                    start=(ib == 0), stop=(ib == NIB - 1),
                    skip_group_check=skip,
                )

            def evac_and_store(g, rb, ps_pair, brep_t):
                out_sb = opool.tile([128, OUT_F], fp16, tag="osb")
                for oc in range(2):
                    nc.vector.tensor_tensor(
                        out=out_sb[:, oc * 512:(oc + 1) * 512],
                        in0=ps_pair[oc][:],
                        in1=brep_t[:, oc * 512:(oc + 1) * 512],
                        op=mybir.AluOpType.add,
                    )
                nc.scalar.dma_start(
                    out=out_ext[g * R + rb * 128:g * R + (rb + 1) * 128, :],
                    in_=out_sb[:],
                )

            # ---- graph 0: chunked streaming with 6-way group interleave ----
            mt_t = mtpool.tile([128, NIB * OUT_F], fp16, tag="mt")
            xt_t = xtpool.tile([128, NIB * R], fp16, tag="xt")
            for ib in range(NIB):
                nc.sync.dma_start(
                    out=mt_t[:, ib * OUT_F:(ib + 1) * OUT_F],
                    in_=mt_ext[0:128, ib * OUT_F:(ib + 1) * OUT_F])
                nc.sync.dma_start(
                    out=xt_t[:, ib * R:(ib + 1) * R],
                    in_=xt_ext[0:128, ib * R:(ib + 1) * R])
            nc.sync.dma_start(out=ones_t[:], in_=ones_ext[:])
            nc.sync.dma_start(out=bmix_t[:], in_=bmix_ext[:])

            ps6 = [psC.tile([128, 512], f32, tag="outps", name=f"ps6_{j}")
                   for j in range(6)]
            for ib in range(NIB):
                for j in range(6):
                    mm_group(ps6[j], xt_t, mt_t, j // 2, j % 2, ib, skip=True)
            brep_t = make_brep(0)
            for rb in range(3):
                evac_and_store(0, rb, ps6[rb * 2:rb * 2 + 2], brep_t)
            for rb in range(3, NRB):
                ps_pair = []
                for oc in range(2):
                    ps = psC.tile([128, 512], f32, tag="outps")
                    for ib in range(NIB):
                        mm_group(ps, xt_t, mt_t, rb, oc, ib)
                    ps_pair.append(ps)
                evac_and_store(0, rb, ps_pair, brep_t)

            # ---- graphs 1..GPC-1: steady-state pipeline ----
            for g in range(1, GPC):
                mt_t = mtpool.tile([128, NIB * OUT_F], fp16, tag="mt")
                nc.sync.dma_start(out=mt_t[:], in_=mt_ext[g * 128:(g + 1) * 128, :])
                xt_t = xtpool.tile([128, NIB * R], fp16, tag="xt")
                nc.sync.dma_start(out=xt_t[:], in_=xt_ext[g * 128:(g + 1) * 128, :])

                brep_t = make_brep(g)
                for rb in range(NRB):
                    ps_pair = []
                    for oc in range(2):
                        ps = psC.tile([128, 512], f32, tag="outps")
                        for ib in range(NIB):
                            mm_group(ps, xt_t, mt_t, rb, oc, ib)
                        ps_pair.append(ps)
                    evac_and_store(g, rb, ps_pair, brep_t)
    nc.compile()
    return nc


def _host_prep(x, coefficients, weight_experts, bias_experts, weight_shared,
               bias_shared):
    x = np.asarray(x)
    coefficients = np.asarray(coefficients)
    weight_experts = np.asarray(weight_experts)
    bias_experts = np.asarray(bias_experts)
    weight_shared = np.asarray(weight_shared)
    bias_shared = np.asarray(bias_shared)
    c32 = coefficients.astype(np.float32)
    # mixed weights [G, O, I] in f32, then to [G, 128(p), NIB, O] fp16
    mw = (c32 @ weight_experts.reshape(E, -1).astype(np.float32)).reshape(
        G, OUT_F, IN_F)
    mw += weight_shared[0]
    # mt[g, p, ib, o] = mw[g, o, ib*128+p]
    mt = np.ascontiguousarray(
        mw.reshape(G, OUT_F, NIB, 128).transpose(0, 3, 2, 1)).astype(np.float16)
    mt = mt.reshape(G * 128, NIB * OUT_F)

    # xt[g, p, ib, r] = x[g*R+r, ib*128+p]
    xt = np.ascontiguousarray(
        x.reshape(G, R, NIB, 128).transpose(0, 3, 2, 1)).astype(np.float16)
    xt = xt.reshape(G * 128, NIB * R)

    bm = (c32 @ bias_experts.astype(np.float32) + bias_shared[0]).astype(
        np.float16)  # [G, OUT_F]
    ones = np.ones((1, 128), dtype=np.float16)

    in_maps = []
    for c in range(NCORES):
        in_maps.append({
            "mt": mt[c * GPC * 128:(c + 1) * GPC * 128],
            "xt": xt[c * GPC * 128:(c + 1) * GPC * 128],
            "bmix": bm[c * GPC:(c + 1) * GPC].reshape(1, GPC * OUT_F),
            "ones": ones,
        })
    return in_maps


def kernel(x, coefficients, weight_experts, bias_experts, weight_shared,
           bias_shared, _want_trace=False):
    if "nc" not in _CACHED:
        _CACHED["nc"] = build_kernel()
    nc = _CACHED["nc"]
    in_maps = _host_prep(x, coefficients, weight_experts, bias_experts,
                         weight_shared, bias_shared)
    kw = {}
    if _want_trace:
        kw = dict(trace=True)
    res = run_bass_kernel_spmd(nc, in_maps, core_ids=list(range(NCORES)), **kw)
    _CACHED["last_result"] = res
    out = np.concatenate(
        [res.results[c]["out"] for c in range(NCORES)], axis=0
    ).astype(np.float32)
    return out


# revision 10
# speedup vs baseline: 1.0542x; 1.0290x over previous
"""Trainium2 Bass kernel for nn_MOLELinear (MoE-style mixed linear layer).

Math (per graph g):
    mixed_w[g] = sum_e coefficients[g, e] * weight_experts[e] + weight_shared[0]
    mixed_b[g] = coefficients[g] @ bias_experts + bias_shared[0]
    out[g]     = x[g] @ mixed_w[g].T + mixed_b[g]

Strategy (8 NeuronCores, data-parallel over graphs; 8 graphs per core):
  * The mix einsum is 0.8% of total FLOPs, and shipping premixed per-graph
    weights costs exactly the same DMA bytes as shipping the expert stack
    (16.8 MB/core either way). So the host premixes mixed_w (and mixed_b),
    pre-transposes both mixed_w and x into the [i-major] operand layout the
    PE needs, and casts to fp16. The device then runs ONLY the main matmul:
    dense back-to-back K=128/M=128/N=512 fp16 matmuls accumulating over 8
    i-blocks in PSUM — no transposes, no mix phase, PE stays HAM-warm.
  * Ramp hiding: graph 0's operands arrive as 16 per-i-block chunk DMAs and
    eight PSUM accumulation groups (rb 0-3 x oc 0-1) are interleaved
    chunk-by-chunk, so the PE computes during the initial HBM transfer
    instead of idling ~17 us.
  * Bias: premixed on host, broadcast across partitions by the otherwise
    idle GpSimd engine (partition_broadcast), added during PSUM->SBUF
    evacuation by DVE tensor_tensor. No PE or PSUM cost.
  * Output is written fp16 (halves output DMA; ~2e-4 extra rel err) and
    cast back to f32 on the host. Output DMAs ride the ACT HWDGE ring so
    they don't queue behind input DMAs on Sync.
"""

import numpy as np

import concourse.bacc as bacc
import concourse.mybir as mybir
import concourse.tile as tile
from concourse.bass_utils import run_bass_kernel_spmd

f32 = mybir.dt.float32
fp16 = mybir.dt.float16

NCORES = 8
G = 64                  # total graphs
GPC = G // NCORES       # graphs per core
R = 1024                # rows per graph
IN_F = 1024
OUT_F = 1024
E = 8                   # routed experts
NIB = IN_F // 128       # i blocks (8)
NRB = R // 128          # row blocks per graph (8)

_CACHED = {}


def build_kernel():
    nc = bacc.Bacc(None, target_bir_lowering=False)

    # host-premixed, transposed operands (SBUF layout, contiguous rows):
    #   mt[g*128+p, ib*OUT_F+o] = mixed_w[g][o, ib*128+p]
    #   xt[g*128+p, ib*R + r]   = x[g*R+r, ib*128+p]
    mt_ext = nc.declare_dram_parameter("mt", [GPC * 128, NIB * OUT_F], fp16,
                                       isOutput=False)
    xt_ext = nc.declare_dram_parameter("xt", [GPC * 128, NIB * R], fp16,
                                       isOutput=False)
    bmix_ext = nc.declare_dram_parameter("bmix", [1, GPC * OUT_F], f32,
                                         isOutput=False)
    out_ext = nc.declare_dram_parameter("out", [GPC * R, OUT_F], fp16,
                                        isOutput=True)

    with tile.TileContext(nc) as tc:
        with (
            tc.tile_pool(name="consts", bufs=1) as cpool,
            tc.tile_pool(name="mt", bufs=2) as mtpool,
            tc.tile_pool(name="xt", bufs=2) as xtpool,
            tc.tile_pool(name="outs", bufs=4) as opool,
            tc.tile_pool(name="brep", bufs=2) as bpool,
            tc.tile_pool(name="psC", bufs=8, space="PSUM") as psC,
        ):
            bmix_t = cpool.tile([1, GPC * OUT_F], f32, tag="bmix")

            def make_brep(g):
                brep_t = bpool.tile([128, OUT_F], f32, tag="brep")
                nc.gpsimd.partition_broadcast(
                    brep_t[:], bmix_t[:, g * OUT_F:(g + 1) * OUT_F])
                return brep_t

            def mm_group(ps, xt_t, mt_t, rb, oc, ib, skip=False):
                nc.tensor.matmul(
                    ps[:],
                    xt_t[:, ib * R + rb * 128:ib * R + (rb + 1) * 128],
                    mt_t[:, ib * OUT_F + oc * 512:ib * OUT_F + (oc + 1) * 512],
                    start=(ib == 0), stop=(ib == NIB - 1),
                    skip_group_check=skip,
                )

            def evac_and_store(g, rb, ps_pair, brep_t):
                out_sb = opool.tile([128, OUT_F], fp16, tag="osb")
                for oc in range(2):
                    nc.vector.tensor_tensor(
                        out=out_sb[:, oc * 512:(oc + 1) * 512],
                        in0=ps_pair[oc][:],
                        in1=brep_t[:, oc * 512:(oc + 1) * 512],
                        op=mybir.AluOpType.add,
                    )
                nc.scalar.dma_start(
                    out=out_ext[g * R + rb * 128:g * R + (rb + 1) * 128, :],
                    in_=out_sb[:],
                )

            # ---- graph 0: chunked streaming with 8-way group interleave ----
            mt_t = mtpool.tile([128, NIB * OUT_F], fp16, tag="mt")
            xt_t = xtpool.tile([128, NIB * R], fp16, tag="xt")
            for ib in range(NIB):
                nc.sync.dma_start(
                    out=mt_t[:, ib * OUT_F:(ib + 1) * OUT_F],
                    in_=mt_ext[0:128, ib * OUT_F:(ib + 1) * OUT_F])
                nc.sync.dma_start(
                    out=xt_t[:, ib * R:(ib + 1) * R],
                    in_=xt_ext[0:128, ib * R:(ib + 1) * R])
            nc.sync.dma_start(out=bmix_t[:], in_=bmix_ext[:])

            ps8 = [psC.tile([128, 512], f32, tag="outps", name=f"ps8_{j}")
                   for j in range(8)]
            for ib in range(NIB):
                for j in range(8):
                    mm_group(ps8[j], xt_t, mt_t, j // 2, j % 2, ib, skip=True)
            brep_t = make_brep(0)
            for rb in range(4):
                evac_and_store(0, rb, ps8[rb * 2:rb * 2 + 2], brep_t)
            for rb in range(4, NRB):
                ps_pair = []
                for oc in range(2):
                    ps = psC.tile([128, 512], f32, tag="outps")
                    for ib in range(NIB):
                        mm_group(ps, xt_t, mt_t, rb, oc, ib)
                    ps_pair.append(ps)
                evac_and_store(0, rb, ps_pair, brep_t)

            # ---- graphs 1..GPC-1: steady-state pipeline ----
            for g in range(1, GPC):
                mt_t = mtpool.tile([128, NIB * OUT_F], fp16, tag="mt")
                nc.sync.dma_start(out=mt_t[:], in_=mt_ext[g * 128:(g + 1) * 128, :])
                xt_t = xtpool.tile([128, NIB * R], fp16, tag="xt")
                nc.sync.dma_start(out=xt_t[:], in_=xt_ext[g * 128:(g + 1) * 128, :])

                brep_t = make_brep(g)
                for rb in range(NRB):
                    ps_pair = []
                    for oc in range(2):
                        ps = psC.tile([128, 512], f32, tag="outps")
                        for ib in range(NIB):
                            mm_group(ps, xt_t, mt_t, rb, oc, ib)
                        ps_pair.append(ps)
                    evac_and_store(g, rb, ps_pair, brep_t)
    nc.compile()
    return nc


def _host_prep(x, coefficients, weight_experts, bias_experts, weight_shared,
               bias_shared):
    x = np.asarray(x)
    coefficients = np.asarray(coefficients)
    weight_experts = np.asarray(weight_experts)
    bias_experts = np.asarray(bias_experts)
    weight_shared = np.asarray(weight_shared)
    bias_shared = np.asarray(bias_shared)
    c32 = coefficients.astype(np.float32)
    # mixed weights [G, O, I] in f32, then to [G, 128(p), NIB, O] fp16
    mw = (c32 @ weight_experts.reshape(E, -1).astype(np.float32)).reshape(
        G, OUT_F, IN_F)
    mw += weight_shared[0]
    # mt[g, p, ib, o] = mw[g, o, ib*128+p]
    mt = np.ascontiguousarray(
        mw.reshape(G, OUT_F, NIB, 128).transpose(0, 3, 2, 1)).astype(np.float16)
    mt = mt.reshape(G * 128, NIB * OUT_F)

    # xt[g, p, ib, r] = x[g*R+r, ib*128+p]
    xt = np.ascontiguousarray(
        x.reshape(G, R, NIB, 128).transpose(0, 3, 2, 1)).astype(np.float16)
    xt = xt.reshape(G * 128, NIB * R)

    bm = (c32 @ bias_experts.astype(np.float32) + bias_shared[0]).astype(
        np.float32)  # [G, OUT_F]

    in_maps = []
    for c in range(NCORES):
        in_maps.append({
            "mt": mt[c * GPC * 128:(c + 1) * GPC * 128],
            "xt": xt[c * GPC * 128:(c + 1) * GPC * 128],
            "bmix": bm[c * GPC:(c + 1) * GPC].reshape(1, GPC * OUT_F),
        })
    return in_maps


def kernel(x, coefficients, weight_experts, bias_experts, weight_shared,
           bias_shared, _want_trace=False):
    if "nc" not in _CACHED:
        _CACHED["nc"] = build_kernel()
    nc = _CACHED["nc"]
    in_maps = _host_prep(x, coefficients, weight_experts, bias_experts,
                         weight_shared, bias_shared)
    kw = {}
    if _want_trace:
        kw = dict(trace=True)
    res = run_bass_kernel_spmd(nc, in_maps, core_ids=list(range(NCORES)), **kw)
    _CACHED["last_result"] = res
    out = np.concatenate(
        [res.results[c]["out"] for c in range(NCORES)], axis=0
    ).astype(np.float32)
    return out


# revision 13
# speedup vs baseline: 1.0597x; 1.0052x over previous
"""Trainium2 Bass kernel for nn_MOLELinear (MoE-style mixed linear layer).

Math (per graph g):
    mixed_w[g] = sum_e coefficients[g, e] * weight_experts[e] + weight_shared[0]
    mixed_b[g] = coefficients[g] @ bias_experts + bias_shared[0]
    out[g]     = x[g] @ mixed_w[g].T + mixed_b[g]

Strategy (8 NeuronCores, data-parallel over graphs; 8 graphs per core):
  * The mix einsum is 0.8% of total FLOPs, and shipping premixed per-graph
    weights costs exactly the same DMA bytes as shipping the expert stack
    (16.8 MB/core either way). So the host premixes mixed_w (and mixed_b),
    pre-transposes both mixed_w and x into the [i-major] operand layout the
    PE needs, and casts to fp16. The device then runs ONLY the main matmul:
    dense back-to-back K=128/M=128/N=512 fp16 matmuls accumulating over 8
    i-blocks in PSUM — no transposes, no mix phase, PE stays HAM-warm.
  * Ramp hiding: graph 0's operands arrive as 16 per-i-block chunk DMAs and
    eight PSUM accumulation groups (rb 0-3 x oc 0-1) are interleaved
    chunk-by-chunk, so the PE computes during the initial HBM transfer
    instead of idling ~17 us.
  * Bias: premixed on host, broadcast across partitions by the otherwise
    idle GpSimd engine (partition_broadcast), added during PSUM->SBUF
    evacuation by DVE tensor_tensor. No PE or PSUM cost.
  * Output is written fp16 (halves output DMA; ~2e-4 extra rel err) and
    cast back to f32 on the host. Output DMAs ride the ACT HWDGE ring so
    they don't queue behind input DMAs on Sync.
"""

import numpy as np

import concourse.bacc as bacc
import concourse.mybir as mybir
import concourse.tile as tile
from concourse.bass_utils import run_bass_kernel_spmd

f32 = mybir.dt.float32
fp16 = mybir.dt.float16

NCORES = 8
G = 64                  # total graphs
GPC = G // NCORES       # graphs per core
R = 1024                # rows per graph
IN_F = 1024
OUT_F = 1024
E = 8                   # routed experts
NIB = IN_F // 128       # i blocks (8)
NRB = R // 128          # row blocks per graph (8)

_CACHED = {}


def build_kernel():
    nc = bacc.Bacc(None, target_bir_lowering=False)

    # host-premixed, transposed operands (SBUF layout, contiguous rows):
    #   mt[g*128+p, ib*OUT_F+o] = mixed_w[g][o, ib*128+p]
    #   xt[g*128+p, ib*R + r]   = x[g*R+r, ib*128+p]
    mt_ext = nc.declare_dram_parameter("mt", [GPC * 128, NIB * OUT_F], fp16,
                                       isOutput=False)
    xt_ext = nc.declare_dram_parameter("xt", [GPC * 128, NIB * R], fp16,
                                       isOutput=False)
    bmix_ext = nc.declare_dram_parameter("bmix", [1, GPC * OUT_F], f32,
                                         isOutput=False)
    out_ext = nc.declare_dram_parameter("out", [GPC * R, OUT_F], fp16,
                                        isOutput=True)

    with tile.TileContext(nc) as tc:
        with (
            tc.tile_pool(name="consts", bufs=1) as cpool,
            tc.tile_pool(name="mt", bufs=2) as mtpool,
            tc.tile_pool(name="xt", bufs=2) as xtpool,
            tc.tile_pool(name="outs", bufs=4) as opool,
            tc.tile_pool(name="brep", bufs=2) as bpool,
            tc.tile_pool(name="psC", bufs=8, space="PSUM") as psC,
        ):
            bmix_t = cpool.tile([1, GPC * OUT_F], f32, tag="bmix")

            def make_brep(g):
                brep_t = bpool.tile([128, OUT_F], f32, tag="brep")
                nc.gpsimd.partition_broadcast(
                    brep_t[:], bmix_t[:, g * OUT_F:(g + 1) * OUT_F])
                return brep_t

            def mm_group(ps, xt_t, mt_t, rb, oc, ib, skip=False):
                nc.tensor.matmul(
                    ps[:],
                    xt_t[:, ib * R + rb * 128:ib * R + (rb + 1) * 128],
                    mt_t[:, ib * OUT_F + oc * 512:ib * OUT_F + (oc + 1) * 512],
                    start=(ib == 0), stop=(ib == NIB - 1),
                    skip_group_check=skip,
                )

            def evac_and_store(g, rb, ps_pair, brep_t, split=False):
                out_sb = opool.tile([128, OUT_F], fp16, tag="osb")
                for oc in range(2):
                    nc.vector.tensor_tensor(
                        out=out_sb[:, oc * 512:(oc + 1) * 512],
                        in0=ps_pair[oc][:],
                        in1=brep_t[:, oc * 512:(oc + 1) * 512],
                        op=mybir.AluOpType.add,
                    )
                    if split:
                        nc.scalar.dma_start(
                            out=out_ext[g * R + rb * 128:g * R + (rb + 1) * 128,
                                        oc * 512:(oc + 1) * 512],
                            in_=out_sb[:, oc * 512:(oc + 1) * 512],
                        )
                if not split:
                    nc.scalar.dma_start(
                        out=out_ext[g * R + rb * 128:g * R + (rb + 1) * 128, :],
                        in_=out_sb[:],
                    )

            # ---- graph 0: chunked streaming with 8-way group interleave ----
            # The interleave (rb 0-3) only reads the low half of each xt
            # chunk, so the rb 4-7 halves are deferred past the ramp-critical
            # window. oc-major group order means the first 4 MMs need only
            # mt chunk 0's low half + xt chunk 0's low half (640 KB).
            mt_t = mtpool.tile([128, NIB * OUT_F], fp16, tag="mt")
            xt_t = xtpool.tile([128, NIB * R], fp16, tag="xt")
            nc.sync.dma_start(out=mt_t[:, 0:512],
                              in_=mt_ext[0:128, 0:512])
            nc.sync.dma_start(out=xt_t[:, 0:512],
                              in_=xt_ext[0:128, 0:512])
            nc.sync.dma_start(out=mt_t[:, 512:OUT_F],
                              in_=mt_ext[0:128, 512:OUT_F])
            for ib in range(1, NIB):
                nc.sync.dma_start(
                    out=mt_t[:, ib * OUT_F:(ib + 1) * OUT_F],
                    in_=mt_ext[0:128, ib * OUT_F:(ib + 1) * OUT_F])
                nc.sync.dma_start(
                    out=xt_t[:, ib * R:ib * R + 512],
                    in_=xt_ext[0:128, ib * R:ib * R + 512])
            nc.sync.dma_start(out=bmix_t[:], in_=bmix_ext[:])
            for ib in range(NIB):
                nc.sync.dma_start(
                    out=xt_t[:, ib * R + 512:(ib + 1) * R],
                    in_=xt_ext[0:128, ib * R + 512:(ib + 1) * R])

            # j -> (rb, oc) oc-major: j=0..3 -> (j, 0); j=4..7 -> (j-4, 1)
            ps8 = [psC.tile([128, 512], f32, tag="outps", name=f"ps8_{j}")
                   for j in range(8)]
            for ib in range(NIB):
                for j in range(8):
                    mm_group(ps8[j], xt_t, mt_t, j % 4, j // 4, ib, skip=True)
            brep_t = make_brep(0)
            for rb in range(4):
                evac_and_store(0, rb, [ps8[rb], ps8[4 + rb]], brep_t)
            for rb in range(4, NRB):
                ps_pair = []
                for oc in range(2):
                    ps = psC.tile([128, 512], f32, tag="outps")
                    for ib in range(NIB):
                        mm_group(ps, xt_t, mt_t, rb, oc, ib)
                    ps_pair.append(ps)
                evac_and_store(0, rb, ps_pair, brep_t)

            # ---- graphs 1..GPC-1: steady-state pipeline ----
            for g in range(1, GPC):
                mt_t = mtpool.tile([128, NIB * OUT_F], fp16, tag="mt")
                nc.sync.dma_start(out=mt_t[:], in_=mt_ext[g * 128:(g + 1) * 128, :])
                xt_t = xtpool.tile([128, NIB * R], fp16, tag="xt")
                nc.sync.dma_start(out=xt_t[:], in_=xt_ext[g * 128:(g + 1) * 128, :])

                brep_t = make_brep(g)
                for rb in range(NRB):
                    ps_pair = []
                    for oc in range(2):
                        ps = psC.tile([128, 512], f32, tag="outps")
                        for ib in range(NIB):
                            mm_group(ps, xt_t, mt_t, rb, oc, ib)
                        ps_pair.append(ps)
                    last = (g == GPC - 1 and rb == NRB - 1)
                    evac_and_store(g, rb, ps_pair, brep_t, split=last)
    nc.compile()
    return nc


def _host_prep(x, coefficients, weight_experts, bias_experts, weight_shared,
               bias_shared):
    x = np.asarray(x)
    coefficients = np.asarray(coefficients)
    weight_experts = np.asarray(weight_experts)
    bias_experts = np.asarray(bias_experts)
    weight_shared = np.asarray(weight_shared)
    bias_shared = np.asarray(bias_shared)
    c32 = coefficients.astype(np.float32)
    # mixed weights [G, O, I] in f32, then to [G, 128(p), NIB, O] fp16
    mw = (c32 @ weight_experts.reshape(E, -1).astype(np.float32)).reshape(
        G, OUT_F, IN_F)
    mw += weight_shared[0]
    # mt[g, p, ib, o] = mw[g, o, ib*128+p]
    mt = np.ascontiguousarray(
        mw.reshape(G, OUT_F, NIB, 128).transpose(0, 3, 2, 1)).astype(np.float16)
    mt = mt.reshape(G * 128, NIB * OUT_F)

    # xt[g, p, ib, r] = x[g*R+r, ib*128+p]
    xt = np.ascontiguousarray(
        x.reshape(G, R, NIB, 128).transpose(0, 3, 2, 1)).astype(np.float16)
    xt = xt.reshape(G * 128, NIB * R)

    bm = (c32 @ bias_experts.astype(np.float32) + bias_shared[0]).astype(
        np.float32)  # [G, OUT_F]

    in_maps = []
    for c in range(NCORES):
        in_maps.append({
            "mt": mt[c * GPC * 128:(c + 1) * GPC * 128],
            "xt": xt[c * GPC * 128:(c + 1) * GPC * 128],
            "bmix": bm[c * GPC:(c + 1) * GPC].reshape(1, GPC * OUT_F),
        })
    return in_maps


def kernel(x, coefficients, weight_experts, bias_experts, weight_shared,
           bias_shared, _want_trace=False):
    if "nc" not in _CACHED:
        _CACHED["nc"] = build_kernel()
    nc = _CACHED["nc"]
    in_maps = _host_prep(x, coefficients, weight_experts, bias_experts,
                         weight_shared, bias_shared)
    kw = {}
    if _want_trace:
        kw = dict(trace=True)
    res = run_bass_kernel_spmd(nc, in_maps, core_ids=list(range(NCORES)), **kw)
    _CACHED["last_result"] = res
    out = np.concatenate(
        [res.results[c]["out"] for c in range(NCORES)], axis=0
    ).astype(np.float32)
    return out


# revision 14
# speedup vs baseline: 1.0606x; 1.0009x over previous
"""Trainium2 Bass kernel for nn_MOLELinear (MoE-style mixed linear layer).

Math (per graph g):
    mixed_w[g] = sum_e coefficients[g, e] * weight_experts[e] + weight_shared[0]
    mixed_b[g] = coefficients[g] @ bias_experts + bias_shared[0]
    out[g]     = x[g] @ mixed_w[g].T + mixed_b[g]

Strategy (8 NeuronCores, data-parallel over graphs; 8 graphs per core):
  * The mix einsum is 0.8% of total FLOPs, and shipping premixed per-graph
    weights costs exactly the same DMA bytes as shipping the expert stack
    (16.8 MB/core either way). So the host premixes mixed_w (and mixed_b),
    pre-transposes both mixed_w and x into the [i-major] operand layout the
    PE needs, and casts to fp16. The device then runs ONLY the main matmul:
    dense back-to-back K=128/M=128/N=512 fp16 matmuls accumulating over 8
    i-blocks in PSUM — no transposes, no mix phase, PE stays HAM-warm.
  * Ramp hiding: graph 0's operands arrive as 16 per-i-block chunk DMAs and
    eight PSUM accumulation groups (rb 0-3 x oc 0-1) are interleaved
    chunk-by-chunk, so the PE computes during the initial HBM transfer
    instead of idling ~17 us.
  * Bias: premixed on host, broadcast across partitions by the otherwise
    idle GpSimd engine (partition_broadcast), added during PSUM->SBUF
    evacuation by DVE tensor_tensor. No PE or PSUM cost.
  * Output is written fp16 (halves output DMA; ~2e-4 extra rel err) and
    cast back to f32 on the host. Output DMAs ride the ACT HWDGE ring so
    they don't queue behind input DMAs on Sync.
"""

import numpy as np

import concourse.bacc as bacc
import concourse.mybir as mybir
import concourse.tile as tile
from concourse.bass_utils import run_bass_kernel_spmd

f32 = mybir.dt.float32
fp16 = mybir.dt.float16

NCORES = 8
G = 64                  # total graphs
GPC = G // NCORES       # graphs per core
R = 1024                # rows per graph
IN_F = 1024
OUT_F = 1024
E = 8                   # routed experts
NIB = IN_F // 128       # i blocks (8)
NRB = R // 128          # row blocks per graph (8)

_CACHED = {}


def build_kernel():
    nc = bacc.Bacc(None, target_bir_lowering=False)

    # host-premixed, transposed operands (SBUF layout, contiguous rows):
    #   mt[g*128+p, ib*OUT_F+o] = mixed_w[g][o, ib*128+p]
    #   xt[g*128+p, ib*R + r]   = x[g*R+r, ib*128+p]
    mt_ext = nc.declare_dram_parameter("mt", [GPC * 128, NIB * OUT_F], fp16,
                                       isOutput=False)
    xt_ext = nc.declare_dram_parameter("xt", [GPC * 128, NIB * R], fp16,
                                       isOutput=False)
    bmix_ext = nc.declare_dram_parameter("bmix", [1, GPC * OUT_F], f32,
                                         isOutput=False)
    out_ext = nc.declare_dram_parameter("out", [GPC * R, OUT_F], fp16,
                                        isOutput=True)

    with tile.TileContext(nc) as tc:
        with (
            tc.tile_pool(name="consts", bufs=1) as cpool,
            tc.tile_pool(name="mt", bufs=2) as mtpool,
            tc.tile_pool(name="xt", bufs=2) as xtpool,
            tc.tile_pool(name="outs", bufs=4) as opool,
            tc.tile_pool(name="brep", bufs=2) as bpool,
            tc.tile_pool(name="psC", bufs=8, space="PSUM") as psC,
        ):
            bmix_t = cpool.tile([1, GPC * OUT_F], f32, tag="bmix")

            def make_brep(g):
                brep_t = bpool.tile([128, OUT_F], f32, tag="brep")
                nc.gpsimd.partition_broadcast(
                    brep_t[:], bmix_t[:, g * OUT_F:(g + 1) * OUT_F])
                return brep_t

            def mm_group(ps, xt_t, mt_t, rb, oc, ib, skip=False):
                nc.tensor.matmul(
                    ps[:],
                    xt_t[:, ib * R + rb * 128:ib * R + (rb + 1) * 128],
                    mt_t[:, ib * OUT_F + oc * 512:ib * OUT_F + (oc + 1) * 512],
                    start=(ib == 0), stop=(ib == NIB - 1),
                    skip_group_check=skip,
                )

            def evac_and_store(g, rb, ps_pair, brep_t, split=False):
                out_sb = opool.tile([128, OUT_F], fp16, tag="osb")
                for oc in range(2):
                    nc.vector.tensor_tensor(
                        out=out_sb[:, oc * 512:(oc + 1) * 512],
                        in0=ps_pair[oc][:],
                        in1=brep_t[:, oc * 512:(oc + 1) * 512],
                        op=mybir.AluOpType.add,
                    )
                    if split:
                        nc.scalar.dma_start(
                            out=out_ext[g * R + rb * 128:g * R + (rb + 1) * 128,
                                        oc * 512:(oc + 1) * 512],
                            in_=out_sb[:, oc * 512:(oc + 1) * 512],
                        )
                if not split:
                    nc.scalar.dma_start(
                        out=out_ext[g * R + rb * 128:g * R + (rb + 1) * 128, :],
                        in_=out_sb[:],
                    )

            # ---- PE warm-up: HAM un-throttles only after ~3.4us of sustained
            # matmul activity. Real matmuls can't start until graph 0's first
            # chunks land (~4.5us after preamble), so burn that window with
            # throwaway matmuls on a zeroed tile — by the time data arrives
            # the PE runs at 2.4GHz instead of 1.2.
            warm_t = cpool.tile([128, 640], fp16, tag="warm")
            nc.gpsimd.memset(warm_t[:], 0.0)
            warm_ps = psC.tile([128, 512], f32, tag="outps", name="warm_ps")
            NWARM = 12
            for k in range(NWARM):
                nc.tensor.matmul(
                    warm_ps[:], warm_t[:, 0:128], warm_t[:, 128:640],
                    start=(k == 0), stop=(k == NWARM - 1),
                    skip_group_check=True)

            # ---- graph 0: chunked streaming with 8-way group interleave ----
            # The interleave (rb 0-3) only reads the low half of each xt
            # chunk, so the rb 4-7 halves are deferred past the ramp-critical
            # window. oc-major group order means the first 4 MMs need only
            # mt chunk 0's low half + xt chunk 0's low half (640 KB).
            mt_t = mtpool.tile([128, NIB * OUT_F], fp16, tag="mt")
            xt_t = xtpool.tile([128, NIB * R], fp16, tag="xt")
            nc.sync.dma_start(out=mt_t[:, 0:512],
                              in_=mt_ext[0:128, 0:512])
            nc.sync.dma_start(out=xt_t[:, 0:512],
                              in_=xt_ext[0:128, 0:512])
            nc.sync.dma_start(out=mt_t[:, 512:OUT_F],
                              in_=mt_ext[0:128, 512:OUT_F])
            for ib in range(1, NIB):
                nc.sync.dma_start(
                    out=mt_t[:, ib * OUT_F:(ib + 1) * OUT_F],
                    in_=mt_ext[0:128, ib * OUT_F:(ib + 1) * OUT_F])
                nc.sync.dma_start(
                    out=xt_t[:, ib * R:ib * R + 512],
                    in_=xt_ext[0:128, ib * R:ib * R + 512])
            nc.sync.dma_start(out=bmix_t[:], in_=bmix_ext[:])
            for ib in range(NIB):
                nc.sync.dma_start(
                    out=xt_t[:, ib * R + 512:(ib + 1) * R],
                    in_=xt_ext[0:128, ib * R + 512:(ib + 1) * R])

            # j -> (rb, oc) oc-major: j=0..3 -> (j, 0); j=4..7 -> (j-4, 1)
            ps8 = [psC.tile([128, 512], f32, tag="outps", name=f"ps8_{j}")
                   for j in range(8)]
            for ib in range(NIB):
                for j in range(8):
                    mm_group(ps8[j], xt_t, mt_t, j % 4, j // 4, ib, skip=True)
            brep_t = make_brep(0)
            for rb in range(4):
                evac_and_store(0, rb, [ps8[rb], ps8[4 + rb]], brep_t)
            for rb in range(4, NRB):
                ps_pair = []
                for oc in range(2):
                    ps = psC.tile([128, 512], f32, tag="outps")
                    for ib in range(NIB):
                        mm_group(ps, xt_t, mt_t, rb, oc, ib)
                    ps_pair.append(ps)
                evac_and_store(0, rb, ps_pair, brep_t)

            # ---- graphs 1..GPC-1: steady-state pipeline ----
            for g in range(1, GPC):
                mt_t = mtpool.tile([128, NIB * OUT_F], fp16, tag="mt")
                nc.sync.dma_start(out=mt_t[:], in_=mt_ext[g * 128:(g + 1) * 128, :])
                xt_t = xtpool.tile([128, NIB * R], fp16, tag="xt")
                nc.sync.dma_start(out=xt_t[:], in_=xt_ext[g * 128:(g + 1) * 128, :])

                brep_t = make_brep(g)
                for rb in range(NRB):
                    ps_pair = []
                    for oc in range(2):
                        ps = psC.tile([128, 512], f32, tag="outps")
                        for ib in range(NIB):
                            mm_group(ps, xt_t, mt_t, rb, oc, ib)
                        ps_pair.append(ps)
                    last = (g == GPC - 1 and rb == NRB - 1)
                    evac_and_store(g, rb, ps_pair, brep_t, split=last)
    nc.compile()
    return nc


def _host_prep(x, coefficients, weight_experts, bias_experts, weight_shared,
               bias_shared):
    x = np.asarray(x)
    coefficients = np.asarray(coefficients)
    weight_experts = np.asarray(weight_experts)
    bias_experts = np.asarray(bias_experts)
    weight_shared = np.asarray(weight_shared)
    bias_shared = np.asarray(bias_shared)
    c32 = coefficients.astype(np.float32)
    # mixed weights [G, O, I] in f32, then to [G, 128(p), NIB, O] fp16
    mw = (c32 @ weight_experts.reshape(E, -1).astype(np.float32)).reshape(
        G, OUT_F, IN_F)
    mw += weight_shared[0]
    # mt[g, p, ib, o] = mw[g, o, ib*128+p]
    mt = np.ascontiguousarray(
        mw.reshape(G, OUT_F, NIB, 128).transpose(0, 3, 2, 1)).astype(np.float16)
    mt = mt.reshape(G * 128, NIB * OUT_F)

    # xt[g, p, ib, r] = x[g*R+r, ib*128+p]
    xt = np.ascontiguousarray(
        x.reshape(G, R, NIB, 128).transpose(0, 3, 2, 1)).astype(np.float16)
    xt = xt.reshape(G * 128, NIB * R)

    bm = (c32 @ bias_experts.astype(np.float32) + bias_shared[0]).astype(
        np.float32)  # [G, OUT_F]

    in_maps = []
    for c in range(NCORES):
        in_maps.append({
            "mt": mt[c * GPC * 128:(c + 1) * GPC * 128],
            "xt": xt[c * GPC * 128:(c + 1) * GPC * 128],
            "bmix": bm[c * GPC:(c + 1) * GPC].reshape(1, GPC * OUT_F),
        })
    return in_maps


def kernel(x, coefficients, weight_experts, bias_experts, weight_shared,
           bias_shared, _want_trace=False):
    if "nc" not in _CACHED:
        _CACHED["nc"] = build_kernel()
    nc = _CACHED["nc"]
    in_maps = _host_prep(x, coefficients, weight_experts, bias_experts,
                         weight_shared, bias_shared)
    kw = {}
    if _want_trace:
        kw = dict(trace=True)
    res = run_bass_kernel_spmd(nc, in_maps, core_ids=list(range(NCORES)), **kw)
    _CACHED["last_result"] = res
    out = np.concatenate(
        [res.results[c]["out"] for c in range(NCORES)], axis=0
    ).astype(np.float32)
    return out
